# revision 47
# baseline (speedup 1.0000x reference)
"""Self-contained Trainium2 Bass kernel for nn_DGAGeoGeneration.

Sharding: 8 cores; core c handles batch b=c//4, L0 query shard j=c%4
(1024 queries). FPS + levels 2/1 are replicated within each 4-core batch
group (device-local, no collectives); level 0 is query-sharded.

Algorithmic structure (validated against the jax reference in numpy):
  - fps(512) is a prefix of fps(1024)  -> one 1024-step FPS per batch.
  - One KNN top-16 per query serves k=16/12/8 (subset-of-ranks property).
  - KNN selection by partial score 2*q.s - |s|^2 (order-equal to distance).
  - FPS distances use the reference's direct (x-c)^2 form (bit-exact argmax).
  - ab2 bias dropped (softmax shift invariance).
"""
import numpy as np
from contextlib import ExitStack

import concourse.bass as bass
import concourse.bass_isa as bass_isa
import concourse.tile as tile
import concourse.mybir as mybir
from concourse import bacc
from concourse.bass import ds
from concourse.bass_utils import run_bass_kernel_spmd
from concourse.masks import make_identity

P = 128
f32 = mybir.dt.float32
i16 = mybir.dt.int16
u32 = mybir.dt.uint32
AL = mybir.AluOpType
AX = mybir.AxisListType
AF = mybir.ActivationFunctionType

B, N, C_IN, DIM = 2, 4096, 128, 64
NQ1, NQ2 = 1024, 512
SHARD = 1024
FPS_UNROLL = 16

_CACHE = {}
DEBUG = False
STAGE = 5
FPS_ITERS = NQ1
FPS_STATIC = False
NO_GATHER = False


def _prep_weights(params):
    w = {}
    for l in range(3):
        p = params["dga"][l]
        g, bta, mu, var = [np.asarray(x, np.float32) for x in p["pbn"]]
        pinv = (g / np.sqrt(var + np.float32(1e-5))).astype(np.float32)
        pshift = (bta - mu * pinv).astype(np.float32)
        ag, abta, amu, avar = [np.asarray(x, np.float32) for x in p["abn"]]
        ainv = (ag / np.sqrt(avar + np.float32(1e-5))).astype(np.float32)
        ashift = (abta - amu * ainv).astype(np.float32)
        w[f"wqT{l}"] = np.ascontiguousarray(np.asarray(p["wq"], np.float32).T)
        w[f"bq{l}"] = np.asarray(p["bq"], np.float32).reshape(-1, 1)
        w[f"wkT{l}"] = np.ascontiguousarray(np.asarray(p["wk"], np.float32).T)
        w[f"bk{l}"] = np.asarray(p["bk"], np.float32).reshape(-1, 1)
        w[f"wvT{l}"] = np.ascontiguousarray(np.asarray(p["wv"], np.float32).T)
        w[f"bv{l}"] = np.asarray(p["bv"], np.float32).reshape(-1, 1)
        w[f"pw1T{l}"] = np.ascontiguousarray(np.asarray(p["pw1"], np.float32).T)
        w[f"pe1s{l}"] = pinv.reshape(-1, 1)
        w[f"pe1b{l}"] = (np.asarray(p["pb1"], np.float32) * pinv + pshift).reshape(-1, 1)
        w[f"pw2T{l}"] = np.ascontiguousarray(np.asarray(p["pw2"], np.float32).T)
        w[f"pb2{l}"] = np.asarray(p["pb2"], np.float32).reshape(-1, 1)
        w[f"aw1T{l}"] = np.ascontiguousarray(np.asarray(p["aw1"], np.float32).T)
        at1s = ainv.reshape(-1, 1)
        at1b = (np.asarray(p["ab1"], np.float32) * ainv + ashift).reshape(-1, 1)
        w[f"at1sa{l}"], w[f"at1sb{l}"] = at1s[0:128].copy(), at1s[128:256].copy()
        w[f"at1ba{l}"], w[f"at1bb{l}"] = at1b[0:128].copy(), at1b[128:256].copy()
        aw2T = np.ascontiguousarray(np.asarray(p["aw2"], np.float32).T)
        w[f"aw2Ta{l}"], w[f"aw2Tb{l}"] = aw2T[0:128].copy(), aw2T[128:256].copy()
        w[f"weT{l}"] = np.ascontiguousarray(np.asarray(p["we"], np.float32).T)
        w[f"be{l}"] = np.asarray(p["be"], np.float32).reshape(-1, 1)
    w["iota_lin"] = (np.arange(32, dtype=np.float32)[None, :]
                     + 32.0 * np.arange(128, dtype=np.float32)[:, None]).copy()
    for i in range(2):
        q = params["qmlp"][i]
        m_w1T = np.ascontiguousarray(np.asarray(q["w1"], np.float32).T)
        w[f"m_w1Ta{i}"], w[f"m_w1Tb{i}"] = m_w1T[0:128].copy(), m_w1T[128:256].copy()
        w[f"m_b1{i}"] = np.asarray(q["b1"], np.float32).reshape(-1, 1)
        w[f"m_w2T{i}"] = np.ascontiguousarray(np.asarray(q["w2"], np.float32).T)
        m_wsT = np.ascontiguousarray(np.asarray(q["ws"], np.float32).T)
        w[f"m_wsTa{i}"], w[f"m_wsTb{i}"] = m_wsT[0:128].copy(), m_wsT[128:256].copy()
        w[f"m_bsum{i}"] = (np.asarray(q["b2"], np.float32)
                           + np.asarray(q["bs"], np.float32)).reshape(-1, 1)
    return w


def _build_program(wnames):
    nc = bacc.Bacc("TRN2", target_bir_lowering=False, debug=False)

    pq_d = nc.dram_tensor("pq", [3, N], f32, kind="ExternalInput")
    fq_d = nc.dram_tensor("fq", [C_IN, N], f32, kind="ExternalInput")
    ps_d = nc.dram_tensor("ps", [3, N], f32, kind="ExternalInput")
    fs_d = nc.dram_tensor("fs", [C_IN, N], f32, kind="ExternalInput")
    pqsh_d = nc.dram_tensor("pq_sh", [3, SHARD], f32, kind="ExternalInput")
    fqsh_d = nc.dram_tensor("fq_sh", [C_IN, SHARD], f32, kind="ExternalInput")
    out_d = nc.dram_tensor("out", [C_IN, SHARD], f32, kind="ExternalOutput")
    dbg = {}
    if DEBUG:
        for nm, shp in [("dbg_fps", [1, NQ1]), ("dbg_idxsh", [P, 128]),
                        ("dbg_idxf", [P, 128]), ("dbg_pqF", [16, NQ1]),
                        ("dbg_fqF", [C_IN, NQ1]), ("dbg_pref2", [C_IN, NQ2]),
                        ("dbg_proj1", [C_IN, NQ1]), ("dbg_f1l1", [C_IN, NQ1]),
                        ("dbg_pref1", [C_IN, NQ1]), ("dbg_proj0", [C_IN, SHARD]),
                        ("dbg_f1l0", [C_IN, SHARD])]:
            dbg[nm] = nc.dram_tensor(nm, shp, f32, kind="ExternalOutput")

    with tile.TileContext(nc) as tc, ExitStack() as ctx:
        const = ctx.enter_context(tc.tile_pool(name="const", bufs=1))
        big = ctx.enter_context(tc.tile_pool(name="big", bufs=1))
        work = ctx.enter_context(tc.tile_pool(name="work", bufs=2))
        dpool = ctx.enter_context(tc.tile_pool(name="dpool", bufs=1))
        gpool = ctx.enter_context(tc.tile_pool(name="gpool", bufs=1))
        lhs_pool = ctx.enter_context(tc.tile_pool(name="lhs", bufs=3))
        fpstmp = ctx.enter_context(tc.tile_pool(name="fpstmp", bufs=2))
        psum = ctx.enter_context(tc.tile_pool(name="psum", bufs=4, space="PSUM"))
        psum1 = ctx.enter_context(tc.tile_pool(name="psum1", bufs=2, space="PSUM"))
        psumT = ctx.enter_context(tc.tile_pool(name="psumT", bufs=2, space="PSUM"))

        wsb = {}
        for name, shape in wnames:
            d = nc.dram_tensor(name, list(shape), f32, kind="ExternalInput")
            t = const.tile(list(shape), f32, name=f"w_{name}")
            nc.sync.dma_start(t[:], d.ap())
            wsb[name] = t

        ident = const.tile([P, P], f32, name="ident")
        make_identity(nc, ident[:])
        ones_row = const.tile([1, P], f32, name="ones_row")
        nc.vector.memset(ones_row[:], 1.0)
        ones3 = const.tile([3, 1], f32, name="ones3")
        nc.vector.memset(ones3[:], 1.0)

        # ---------------- loads ----------------
        fq = big.tile([C_IN, N], f32, tag="keyl", name="fq")
        nc.sync.dma_start(fq[:], fq_d.ap())
        ps16 = const.tile([16, N], f32)     # rows 0-2 ps, row 3 = -|s|^2
        nc.vector.memset(ps16[:], 0.0)
        nc.sync.dma_start(ps16[0:3, :], ps_d.ap())
        pq16 = dpool.tile([16, N], f32, tag="D", name="pq16")
        nc.vector.memset(pq16[:], 0.0)
        nc.sync.dma_start(pq16[0:3, :], pq_d.ap())
        pqsh = const.tile([3, SHARD], f32)
        nc.sync.dma_start(pqsh[:], pqsh_d.ap())
        fqsh = big.tile([C_IN, SHARD], f32, tag="fqsh")
        nc.sync.dma_start(fqsh[:], fqsh_d.ap())

        def sqsum_chunks(src3, n, nm, neg_dst=None, qq_dst=None):
            """Per 512-col chunks: r = x^2+y^2+z^2 of src3 rows 0:3.
            neg_dst: [1, n] row to receive -r (via DMA); qq_dst: [128, n/128]
            per-tile scalar layout filled via a DRAM bounce."""
            qq_dram = (nc.dram_tensor(f"qqscr_{nm}", [1, n], f32)
                       if qq_dst is not None else None)
            for c0 in range(0, n, 512):
                sq = work.tile([3, 512], f32, tag="sqc", bufs=1)
                nc.vector.tensor_tensor(out=sq[:], in0=src3[0:3, c0:c0 + 512],
                                        in1=src3[0:3, c0:c0 + 512], op=AL.mult)
                ptr = psumT.tile([P, 512], f32, tag="pst")
                nc.tensor.matmul(ptr[0:1, :], ones3[:], sq[:], start=True, stop=True)
                red = work.tile([1, 512], f32, tag="redc", bufs=1)
                nc.vector.tensor_copy(red[:], ptr[0:1, :])
                if qq_dram is not None:
                    nc.sync.dma_start(qq_dram.ap()[0:1, c0:c0 + 512], red[:])
                if neg_dst is not None:
                    nc.vector.tensor_scalar(out=red[:], in0=red[:], scalar1=-1.0,
                                            scalar2=None, op0=AL.mult)
                    nc.sync.dma_start(neg_dst[0:1, c0:c0 + 512], red[:])
            if qq_dst is not None:
                nc.sync.dma_start(
                    qq_dst[:],
                    qq_dram.ap()[0:1, :].rearrange("o (b a) -> (o a) b", a=P))

        # KNN rhs row 3: -|s|^2  (DMA row copy avoids cross-partition ALU)
        sqsum_chunks(ps16, N, "s", neg_dst=ps16[3:4, :])

        # ---------------- FPS ----------------
        fx = const.tile([P, 32], f32)
        fy = const.tile([P, 32], f32)
        fz = const.tile([P, 32], f32)
        nc.sync.dma_start(fx[:], pq_d.ap()[0:1, :].rearrange("o (a b) -> (o a) b", a=P))
        nc.sync.dma_start(fy[:], pq_d.ap()[1:2, :].rearrange("o (a b) -> (o a) b", a=P))
        nc.sync.dma_start(fz[:], pq_d.ap()[2:3, :].rearrange("o (a b) -> (o a) b", a=P))
        xyzi = const.tile([P, 4, 32], f32)
        nc.vector.tensor_scalar(out=xyzi[:, 0, :], in0=fx[:], scalar1=-1.0, scalar2=None, op0=AL.mult)
        nc.vector.tensor_scalar(out=xyzi[:, 1, :], in0=fy[:], scalar1=-1.0, scalar2=None, op0=AL.mult)
        nc.vector.tensor_scalar(out=xyzi[:, 2, :], in0=fz[:], scalar1=-1.0, scalar2=None, op0=AL.mult)
        nc.vector.tensor_copy(xyzi[:, 3, :], wsb["iota_lin"][:])

        dists = const.tile([P, 32], f32)
        nc.vector.memset(dists[:], 1e10)
        fps_row = const.tile([1, NQ1 + FPS_UNROLL], f32)
        nc.vector.memset(fps_row[:], 0.0)
        s4 = const.tile([P, 4], f32)         # (-cx,-cy,-cz, lin)
        cur0 = const.tile([1, 4], f32)
        nc.sync.dma_start(cur0[0:1, 0:3], pq_d.ap()[:, 0:1].rearrange("a o -> o a"))
        nc.vector.tensor_scalar(out=cur0[0:1, 0:3], in0=cur0[0:1, 0:3], scalar1=-1.0,
                                scalar2=None, op0=AL.mult)
        nc.vector.memset(cur0[0:1, 3:4], 0.0)
        nc.gpsimd.partition_broadcast(s4[:], cur0[:])

        def fps_step(t_ap):
            xc3 = fpstmp.tile([P, 3, 32], f32, tag="xc")
            sq = fpstmp.tile([P, 3, 32], f32, tag="sq")
            nc.vector.tensor_scalar(out=xc3[:, 0, :], in0=fx[:], scalar1=s4[:, 0:1], scalar2=None, op0=AL.add)
            nc.vector.tensor_scalar(out=xc3[:, 1, :], in0=fy[:], scalar1=s4[:, 1:2], scalar2=None, op0=AL.add)
            nc.vector.tensor_scalar(out=xc3[:, 2, :], in0=fz[:], scalar1=s4[:, 2:3], scalar2=None, op0=AL.add)
            nc.vector.tensor_tensor(out=sq[:], in0=xc3[:], in1=xc3[:], op=AL.mult)
            d2 = fpstmp.tile([P, 32], f32, tag="d2")
            nc.vector.tensor_reduce(out=d2[:], in_=sq[:].rearrange("p a b -> p b a"),
                                    axis=AX.X, op=AL.add)
            nc.vector.tensor_tensor(out=dists[:], in0=d2[:], in1=dists[:], op=AL.min)
            rm = fpstmp.tile([P, 1], f32, tag="rm")
            nc.vector.tensor_reduce(out=rm[:], in_=dists[:], axis=AX.X, op=AL.max)
            gmb = fpstmp.tile([P, 1], f32, tag="gmb")
            nc.gpsimd.partition_all_reduce(gmb[:], rm[:], channels=P,
                                           reduce_op=bass_isa.ReduceOp.max)
            mm = fpstmp.tile([P, 4, 32], f32, tag="mm")
            nc.vector.scalar_tensor_tensor(
                out=mm[:],
                in0=dists[:].rearrange("p (o f) -> p o f", o=1).to_broadcast([P, 4, 32]),
                scalar=gmb[:, 0:1], in1=xyzi[:], op0=AL.is_equal, op1=AL.mult)
            s4r = fpstmp.tile([P, 4], f32, tag="s4r")
            nc.vector.tensor_reduce(out=s4r[:], in_=mm[:], axis=AX.X, op=AL.add)
            nc.gpsimd.partition_all_reduce(s4[:], s4r[:], channels=P,
                                           reduce_op=bass_isa.ReduceOp.add)
            nc.scalar.copy(fps_row[0:1, t_ap], s4[0:1, 3:4])

        if STAGE >= 1:
            if FPS_STATIC:
                for t in range(1, FPS_ITERS + 1):
                    fps_step(slice(t, t + 1))
            else:
                with tc.For_i(1, FPS_ITERS + 1, FPS_UNROLL) as tv:
                    for u in range(FPS_UNROLL):
                        fps_step(ds(tv + u, 1))

        # wrapped idx of fps columns: W[j%16, j//16] = fps[j]
        fps_i16 = const.tile([1, NQ1], i16)
        nc.vector.tensor_copy(fps_i16[:], fps_row[0:1, 0:NQ1])
        wfps = const.tile([P, NQ1 // 16], i16)
        fiv = fps_i16[0:1, :].rearrange("o (c p) -> o c p", p=16)
        for pp in range(16):
            nc.sync.dma_start(wfps[pp:pp + 1, :], fiv[:, :, pp])
        for g in range(1, 8):
            nc.sync.dma_start(wfps[16 * g:16 * g + 16, :], wfps[0:16, :])

        if DEBUG:
            nc.sync.dma_start(dbg["dbg_fps"].ap(), fps_row[0:1, 0:NQ1])
        if STAGE == 0:
            ztmp0 = big.tile([C_IN, SHARD], f32, tag="tagC", name="zout0")
            nc.vector.memset(ztmp0[:], 0.0)
            nc.vector.tensor_copy(ztmp0[0:16, 0:N // 16], ps16[:].rearrange("p (a b) -> p a b", a=1)[:, 0, 0:N // 16])
            nc.sync.dma_start(out_d.ap(), ztmp0[:])
        pqF16 = const.tile([16, NQ1], f32)
        if STAGE >= 1 and not NO_GATHER:
            nc.gpsimd.ap_gather(pqF16[:], pq16[:], wfps[0:16, :],
                                channels=16, num_elems=N, d=1, num_idxs=NQ1)
        elif STAGE >= 1:
            nc.vector.tensor_copy(pqF16[:], pq16[:, 0:NQ1])
        fqF = big.tile([C_IN, NQ1], f32, tag="fqF")
        if not NO_GATHER:
            nc.gpsimd.ap_gather(fqF[:], fq[:], wfps[:], channels=128,
                                num_elems=N, d=1, num_idxs=NQ1)
        else:
            nc.vector.tensor_copy(fqF[:], fq[:, 0:NQ1])

        # |q|^2 rows -> [128, ntiles] per-tile scalar layout; and pqF16 row 3 = -|q|^2
        qq_f = const.tile([P, NQ1 // P], f32)
        sqsum_chunks(pqF16, NQ1, "f", neg_dst=pqF16[3:4, :], qq_dst=qq_f)
        qq_sh = const.tile([P, SHARD // P], f32)
        sqsum_chunks(pqsh, SHARD, "q", qq_dst=qq_sh)

        # ---------------- KNN top-16 ----------------
        def lhsT_from(src3, t):
            lt = lhs_pool.tile([4, P], f32, tag="lhsT")
            nc.vector.tensor_scalar(out=lt[0:3, :], in0=src3[0:3, P * t:P * t + P],
                                    scalar1=2.0, scalar2=None, op0=AL.mult)
            nc.sync.dma_start(lt[3:4, :], ones_row[:])
            return lt

        def knn_tiles(src3, ntiles, idxf_store, nm):
            for t in range(ntiles):
                lt = lhsT_from(src3, t)
                D = dpool.tile([P, N], f32, tag="D", name=f"D_{nm}_{t}")
                for c0 in range(0, N, 512):
                    pt = psum1.tile([P, 512], f32, tag="ps512b")
                    nc.tensor.matmul(pt[:], lt, ps16[0:4, c0:c0 + 512],
                                     start=True, stop=True)
                    nc.scalar.copy(D[:, c0:c0 + 512], pt[:])
                m1 = work.tile([P, 8], f32, tag="m1")
                m2 = work.tile([P, 8], f32, tag="m2")
                i1 = work.tile([P, 8], u32, tag="i1")
                i2 = work.tile([P, 8], u32, tag="i2")
                nc.vector.max(m1[:], D[:])
                nc.vector.max_index(i1[:], m1[:], D[:])
                nc.vector.match_replace(D[:], m1[:], D[:], -1e30)
                nc.vector.max(m2[:], D[:])
                nc.vector.max_index(i2[:], m2[:], D[:])
                nc.vector.tensor_copy(idxf_store[:, 16 * t:16 * t + 8], i1[:])
                nc.vector.tensor_copy(idxf_store[:, 16 * t + 8:16 * t + 16], i2[:])

        idxf_sh = const.tile([P, 16 * (SHARD // P)], f32)
        idxf_f = const.tile([P, 16 * (NQ1 // P)], f32)
        if STAGE >= 2:
            knn_tiles(pqsh, SHARD // P, idxf_sh, "sh")
            knn_tiles(pqF16, NQ1 // P, idxf_f, "f")
        if DEBUG:
            nc.sync.dma_start(dbg["dbg_idxsh"].ap(), idxf_sh[:])
            nc.sync.dma_start(dbg["dbg_idxf"].ap(), idxf_f[:])
            nc.sync.dma_start(dbg["dbg_pqF"].ap(), pqF16[:])
            nc.sync.dma_start(dbg["dbg_fqF"].ap(), fqF[:])

        def make_wrapped(idxf_store, ntiles, nm):
            ws = []
            for t in range(ntiles):
                pt = psumT.tile([P, P], f32, tag="pst")
                nc.tensor.transpose(pt[0:16, :], idxf_store[:, 16 * t:16 * t + 16], ident)
                w16 = work.tile([16, P], i16, tag="w16")
                nc.vector.tensor_copy(w16[:], pt[0:16, :])
                wfull = const.tile([P, P], i16, name=f"wf_{nm}_{t}")
                for g in range(8):
                    nc.sync.dma_start(wfull[16 * g:16 * g + 16, :], w16[:])
                ws.append(wfull)
            return ws
        if STAGE >= 2:
            w_sh = make_wrapped(idxf_sh, SHARD // P, "sh")
            w_f = make_wrapped(idxf_f, NQ1 // P, "f")

        # ---------------- DGA level ----------------
        def dga_level(l, nq, k, f1, pos1, wlist, out_t):
            key = big.tile([DIM, N], f32, tag="keyl", name=f"key{l}")
            val = big.tile([DIM, N], f32, tag="vall", name=f"val{l}")
            qv = big.tile([DIM, nq], f32, tag="qvl", name=f"qv{l}")
            for dst, wT, bb in ((key, wsb[f"wkT{l}"], wsb[f"bk{l}"]),
                                (val, wsb[f"wvT{l}"], wsb[f"bv{l}"])):
                for c0 in range(0, N, 512):
                    fsch = work.tile([C_IN, 512], f32, tag="fsch")
                    nc.sync.dma_start(fsch[:], fs_d.ap()[:, c0:c0 + 512])
                    pt = psum.tile([P, 512], f32, tag="ps512")
                    nc.tensor.matmul(pt[:DIM, :], wT[:], fsch[:],
                                     start=True, stop=True)
                    nc.scalar.activation(dst[:, c0:c0 + 512], pt[:DIM, :], AF.Identity,
                                         bias=bb[:, 0:1])
            for c0 in range(0, nq, 512):
                cn = min(512, nq - c0)
                pt = psum.tile([P, 512], f32, tag="ps512")
                nc.tensor.matmul(pt[:DIM, :cn], wsb[f"wqT{l}"][:], f1[:, c0:c0 + cn],
                                 start=True, stop=True)
                nc.scalar.activation(qv[:, c0:c0 + cn], pt[:DIM, :cn], AF.Identity,
                                     bias=wsb[f"bq{l}"][:, 0:1])

            agg = big.tile([DIM, nq], f32, tag="aggl", name=f"agg{l}")
            G = 64 * 16
            for ch in range(nq // 64):
                t, h = ch // 2, ch % 2
                wfull = wlist[t]
                widx = wfull[:, 64 * h:64 * h + 64]
                keyg = gpool.tile([DIM, G], f32, tag="keyg")
                valg = gpool.tile([DIM, G], f32, tag="valg")
                psg = gpool.tile([16, G], f32, tag="psg")
                nc.gpsimd.ap_gather(keyg[:], key[:], widx[0:DIM, :],
                                    channels=DIM, num_elems=N, d=1, num_idxs=G)
                nc.gpsimd.ap_gather(valg[:], val[:], widx[0:DIM, :],
                                    channels=DIM, num_elems=N, d=1, num_idxs=G)
                nc.gpsimd.ap_gather(psg[:], ps16[:], widx[0:16, :],
                                    channels=16, num_elems=N, d=1, num_idxs=G)
                pe_sb = gpool.tile([DIM, G], f32, tag="pe_sb")
                att = gpool.tile([DIM, G], f32, tag="att")
                for c0 in range(0, G, 512):
                    sl = slice(c0, c0 + 512)
                    q0 = 64 * ch + 32 * (c0 // 512)   # 32 queries per 512-col sub
                    qs = qv[:, q0:q0 + 32]
                    prels = work.tile([3, 512], f32, tag="prels")
                    nc.vector.tensor_tensor(
                        out=prels[:].rearrange("p (a b) -> p a b", b=16),
                        in0=pos1[0:3, q0:q0 + 32]
                            .rearrange("p (a o) -> p a o", o=1).to_broadcast([3, 32, 16]),
                        in1=psg[0:3, sl].rearrange("p (a b) -> p a b", b=16),
                        op=AL.subtract)
                    qks = work.tile([DIM, 512], f32, tag="qks")
                    nc.vector.tensor_tensor(
                        out=qks[:].rearrange("p (a b) -> p a b", b=16),
                        in0=qs.rearrange("p (a o) -> p a o", o=1).to_broadcast([DIM, 32, 16]),
                        in1=keyg[:, sl].rearrange("p (a b) -> p a b", b=16),
                        op=AL.subtract)
                    pt = psum.tile([P, 512], f32, tag="ps512")
                    nc.tensor.matmul(pt[:DIM, :], wsb[f"pw1T{l}"][:], prels[:],
                                     start=True, stop=True)
                    z = work.tile([DIM, 512], f32, tag="zz")
                    nc.scalar.activation(z[:], pt[:DIM, :], AF.Relu,
                                         bias=wsb[f"pe1b{l}"][:, 0:1],
                                         scale=wsb[f"pe1s{l}"][:, 0:1])
                    pt2 = psum.tile([P, 512], f32, tag="ps512")
                    nc.tensor.matmul(pt2[:DIM, :], wsb[f"pw2T{l}"][:], z[:],
                                     start=True, stop=True)
                    nc.scalar.activation(pe_sb[:, sl], pt2[:DIM, :], AF.Identity,
                                         bias=wsb[f"pb2{l}"][:, 0:1])
                    apre = work.tile([DIM, 512], f32, tag="apre")
                    nc.vector.tensor_tensor(out=apre[:], in0=qks[:], in1=pe_sb[:, sl], op=AL.add)
                    ptA = psum.tile([P, 512], f32, tag="ps512")
                    nc.tensor.matmul(ptA[:], wsb[f"aw1T{l}"][:, 0:128], apre[:],
                                     start=True, stop=True)
                    h0 = work.tile([P, 512], f32, tag="h0")
                    nc.scalar.activation(h0[:], ptA[:], AF.Relu,
                                         bias=wsb[f"at1ba{l}"][:, 0:1],
                                         scale=wsb[f"at1sa{l}"][:, 0:1])
                    ptB = psum.tile([P, 512], f32, tag="ps512")
                    nc.tensor.matmul(ptB[:], wsb[f"aw1T{l}"][:, 128:256], apre[:],
                                     start=True, stop=True)
                    h1 = work.tile([P, 512], f32, tag="h1")
                    nc.scalar.activation(h1[:], ptB[:], AF.Relu,
                                         bias=wsb[f"at1bb{l}"][:, 0:1],
                                         scale=wsb[f"at1sb{l}"][:, 0:1])
                    ptC = psum.tile([P, 512], f32, tag="ps512")
                    nc.tensor.matmul(ptC[:DIM, :], wsb[f"aw2Ta{l}"][:], h0[:],
                                     start=True, stop=False)
                    nc.tensor.matmul(ptC[:DIM, :], wsb[f"aw2Tb{l}"][:], h1[:],
                                     start=False, stop=True)
                    nc.scalar.copy(att[:, sl], ptC[:DIM, :])
                # softmax over first k of 16 slots, then aggregate
                a3 = att[:].rearrange("p (a b) -> p a b", b=16)[:, :, 0:k]
                nc.scalar.activation(a3, a3, AF.Exp)
                asum = work.tile([DIM, 64], f32, tag="asum")
                nc.vector.tensor_reduce(out=asum[:], in_=a3, axis=AX.X, op=AL.add)
                arec = work.tile([DIM, 64], f32, tag="arec")
                nc.vector.reciprocal(arec[:], asum[:])
                nc.vector.tensor_tensor(
                    out=a3, in0=a3,
                    in1=arec[:].rearrange("p (a o) -> p a o", o=1).to_broadcast([DIM, 64, k]),
                    op=AL.mult)
                nc.vector.tensor_tensor(out=valg[:], in0=valg[:], in1=pe_sb[:], op=AL.add)
                av = valg[:].rearrange("p (a b) -> p a b", b=16)[:, :, 0:k]
                nc.vector.tensor_tensor(out=av, in0=a3, in1=av, op=AL.mult)
                nc.vector.tensor_reduce(out=agg[:, 64 * ch:64 * ch + 64], in_=av,
                                        axis=AX.X, op=AL.add)

            for c0 in range(0, nq, 512):
                cn = min(512, nq - c0)
                pt = psum.tile([P, 512], f32, tag="ps512")
                nc.tensor.matmul(pt[:, :cn], wsb[f"weT{l}"][:], agg[:, c0:c0 + cn],
                                 start=True, stop=True)
                nc.vector.scalar_tensor_tensor(
                    out=out_t[:, c0:c0 + cn], in0=pt[:, :cn], scalar=wsb[f"be{l}"][:, 0:1],
                    in1=f1[:, c0:c0 + cn], op0=AL.add, op1=AL.add)

        # ---------------- three_inter ----------------
        def three_inter(nq, m, qsrc4, qq_tiles, msrc4, mfeat, mi, out_t):
            """qsrc4/msrc4: [>=4, *] rows 0-2 coords (msrc4 row 3 = -|m|^2).
            Processed in halves of nq//2 queries to bound SBUF."""
            nh = nq // 2
            nth = nh // P
            for half in range(2):
                tb = half * nth
                wall = work.tile([P, 3 * nth], f32, tag="wall", name=f"wall{mi}_{half}")
                i3all = work.tile([P, 3 * nth], f32, tag="i3all", name=f"i3all{mi}_{half}")
                for tt in range(nth):
                    t = tb + tt
                    lt = lhsT_from(qsrc4, t)
                    Dm = dpool.tile([P, N], f32, tag="D", name=f"Dm{mi}_{t}")
                    for c0 in range(0, m, 512):
                        pt = psum1.tile([P, 512], f32, tag="ps512b")
                        nc.tensor.matmul(pt[:], lt, msrc4[0:4, c0:c0 + 512],
                                         start=True, stop=True)
                        nc.scalar.copy(Dm[:, c0:c0 + 512], pt[:])
                    m8 = work.tile([P, 8], f32, tag="m1")
                    i8 = work.tile([P, 8], u32, tag="i1")
                    nc.vector.max(m8[:], Dm[:, 0:m])
                    nc.vector.max_index(i8[:], m8[:], Dm[:, 0:m])
                    d3 = work.tile([P, 3], f32, tag="d3")
                    nc.vector.scalar_tensor_tensor(
                        out=d3[:], in0=m8[:, 0:3], scalar=-1.0,
                        in1=qq_tiles[:, t:t + 1].to_broadcast([P, 3]),
                        op0=AL.mult, op1=AL.add)
                    nc.vector.tensor_scalar(out=d3[:], in0=d3[:], scalar1=1e-10,
                                            scalar2=None, op0=AL.max)
                    rec = work.tile([P, 3], f32, tag="rec")
                    nc.vector.reciprocal(rec[:], d3[:])
                    rsum = work.tile([P, 1], f32, tag="rsum")
                    nc.vector.tensor_reduce(out=rsum[:], in_=rec[:], axis=AX.X, op=AL.add)
                    rrec = work.tile([P, 1], f32, tag="rrec")
                    nc.vector.reciprocal(rrec[:], rsum[:])
                    nc.vector.tensor_scalar(out=wall[:, 3 * tt:3 * tt + 3], in0=rec[:],
                                            scalar1=rrec[:, 0:1], scalar2=None, op0=AL.mult)
                    nc.vector.tensor_copy(i3all[:, 3 * tt:3 * tt + 3], i8[:, 0:3])
                ptw = psumT.tile([P, P], f32, tag="pst")
                nc.tensor.transpose(ptw[0:3 * nth, :], wall[:, 0:3 * nth], ident)
                wrows = work.tile([P, P], f32, tag="wrows", name=f"wrows{mi}_{half}")
                nc.vector.tensor_copy(wrows[0:3 * nth, :], ptw[0:3 * nth, :])
                pti = psumT.tile([P, P], f32, tag="pst")
                nc.tensor.transpose(pti[0:3 * nth, :], i3all[:, 0:3 * nth], ident)
                irows = work.tile([P, P], i16, tag="irows", name=f"irows{mi}_{half}")
                nc.vector.tensor_copy(irows[0:3 * nth, :], pti[0:3 * nth, :])
                # shuffle rows [3tt+j, c] -> free layout [1, (tt c j)]
                wfree = gpool.tile([1, nh * 3], f32, tag="keyg", name=f"wfree{mi}_{half}")
                ifree = gpool.tile([1, nh * 3], i16, tag="valg", name=f"ifree{mi}_{half}")
                ov = wfree[0:1, :].rearrange("o (t c j) -> o t j c", t=nth, j=3)
                oi = ifree[0:1, :].rearrange("o (t c j) -> o t j c", t=nth, j=3)
                for tt in range(nth):
                    for jj in range(3):
                        nc.sync.dma_start(ov[:, tt, jj, :], wrows[3 * tt + jj:3 * tt + jj + 1, :])
                        nc.sync.dma_start(oi[:, tt, jj, :], irows[3 * tt + jj:3 * tt + jj + 1, :])
                wrap3 = work.tile([P, nh * 3 // 16], i16, tag="wrap3", name=f"wrap3{mi}_{half}")
                ifv = ifree[0:1, :].rearrange("o (c p) -> o c p", p=16)
                for pp in range(16):
                    nc.sync.dma_start(wrap3[pp:pp + 1, :], ifv[:, :, pp])
                for g in range(1, 8):
                    nc.sync.dma_start(wrap3[16 * g:16 * g + 16, :], wrap3[0:16, :])
                g3 = gpool.tile([C_IN, nh * 3], f32, tag="pe_sb", name=f"g3_{mi}_{half}")
                nc.gpsimd.ap_gather(g3[:], mfeat[:], wrap3[:], channels=128,
                                    num_elems=m, d=1, num_idxs=nh * 3)
                for sg in range(0, nh * 3, 512):
                    sn = min(512, nh * 3 - sg)
                    ptb = psum.tile([P, 512], f32, tag="ps512")
                    nc.tensor.matmul(ptb[:, :sn], ones_row[:],
                                     wfree[0:1, sg:sg + sn], start=True, stop=True)
                    nc.vector.tensor_tensor(out=g3[:, sg:sg + sn], in0=g3[:, sg:sg + sn],
                                            in1=ptb[:, :sn], op=AL.mult)
                nc.vector.tensor_reduce(
                    out=out_t[:, half * nh:half * nh + nh],
                    in_=g3[:].rearrange("p (a b) -> p a b", b=3),
                    axis=AX.X, op=AL.add)

        def mlp_res(mi, f_base, proj, nq, out_t):
            h = big.tile([C_IN, nq], f32, tag="tagD", name=f"mlph{mi}")
            for c0 in range(0, nq, 512):
                cn = min(512, nq - c0)
                pt = psum.tile([P, 512], f32, tag="ps512")
                nc.tensor.matmul(pt[:, :cn], wsb[f"m_w1Ta{mi}"][:], f_base[:, c0:c0 + cn],
                                 start=True, stop=False)
                nc.tensor.matmul(pt[:, :cn], wsb[f"m_w1Tb{mi}"][:], proj[:, c0:c0 + cn],
                                 start=False, stop=True)
                nc.scalar.activation(h[:, c0:c0 + cn], pt[:, :cn], AF.Relu,
                                     bias=wsb[f"m_b1{mi}"][:, 0:1])
            for c0 in range(0, nq, 512):
                cn = min(512, nq - c0)
                pt = psum.tile([P, 512], f32, tag="ps512")
                nc.tensor.matmul(pt[:, :cn], wsb[f"m_w2T{mi}"][:], h[:, c0:c0 + cn],
                                 start=True, stop=False)
                nc.tensor.matmul(pt[:, :cn], wsb[f"m_wsTa{mi}"][:], f_base[:, c0:c0 + cn],
                                 start=False, stop=False)
                nc.tensor.matmul(pt[:, :cn], wsb[f"m_wsTb{mi}"][:], proj[:, c0:c0 + cn],
                                 start=False, stop=True)
                nc.vector.tensor_scalar(out=out_t[:, c0:c0 + cn], in0=pt[:, :cn],
                                        scalar1=wsb[f"m_bsum{mi}"][:, 0:1],
                                        scalar2=None, op0=AL.add)

        # ---------------- levels ----------------
        if STAGE <= 2:
            ztmp = big.tile([C_IN, SHARD], f32, tag="tagC", name="zout")
            nc.vector.memset(ztmp[:], 0.0)
            nc.vector.tensor_copy(ztmp[:, 0:NQ1], fqF[:])
            nc.sync.dma_start(out_d.ap(), ztmp[:])
        if STAGE >= 3:
            pre_f2 = big.tile([C_IN, NQ2], f32, tag="tagA", name="pre_f2")
            dga_level(2, NQ2, 8, fqF[:, 0:NQ2], pqF16[0:3, 0:NQ2], w_f[0:4], pre_f2)
            if DEBUG:
                nc.sync.dma_start(dbg["dbg_pref2"].ap(), pre_f2[:])
        if STAGE == 3:
            ztmp = big.tile([C_IN, SHARD], f32, tag="tagC", name="zout")
            nc.vector.memset(ztmp[:], 0.0)
            nc.vector.tensor_copy(ztmp[:, 0:NQ2], pre_f2[:])
            nc.sync.dma_start(out_d.ap(), ztmp[:])
        if STAGE >= 4:
            proj1 = big.tile([C_IN, NQ1], f32, tag="tagB", name="proj1")
            three_inter(NQ1, NQ2, pqF16, qq_f, pqF16[:, 0:NQ2], pre_f2, 1, proj1)
            if DEBUG:
                nc.sync.dma_start(dbg["dbg_proj1"].ap(), proj1[:])
            f1_l1 = big.tile([C_IN, NQ1], f32, tag="tagC", name="f1_l1")
            mlp_res(1, fqF, proj1, NQ1, f1_l1)
            if DEBUG:
                nc.sync.dma_start(dbg["dbg_f1l1"].ap(), f1_l1[:])
            pre_f1 = big.tile([C_IN, NQ1], f32, tag="tagD", name="pre_f1")
            dga_level(1, NQ1, 12, f1_l1, pqF16[0:3, :], w_f, pre_f1)
            if DEBUG:
                nc.sync.dma_start(dbg["dbg_pref1"].ap(), pre_f1[:])
        if STAGE == 4:
            ztmp = big.tile([C_IN, SHARD], f32, tag="tagC", name="zout")
            nc.vector.tensor_copy(ztmp[:], pre_f1[:])
            nc.sync.dma_start(out_d.ap(), ztmp[:])
        if STAGE >= 5:
            proj0 = big.tile([C_IN, SHARD], f32, tag="tagA", name="proj0")
            three_inter(SHARD, NQ1, pqsh, qq_sh, pqF16, pre_f1, 0, proj0)
            if DEBUG:
                nc.sync.dma_start(dbg["dbg_proj0"].ap(), proj0[:])
            f1_l0 = big.tile([C_IN, SHARD], f32, tag="tagB", name="f1_l0")
            mlp_res(0, fqsh, proj0, SHARD, f1_l0)
            if DEBUG:
                nc.sync.dma_start(dbg["dbg_f1l0"].ap(), f1_l0[:])
            out_sb = big.tile([C_IN, SHARD], f32, tag="tagC", name="out_sb")
            dga_level(0, SHARD, 16, f1_l0, pqsh[0:3, :], w_sh, out_sb)

            nc.sync.dma_start(out_d.ap(), out_sb[:])

    nc.finalize()
    return nc


def kernel(pq, fq, ps, fs, params):
    pq = np.ascontiguousarray(np.asarray(pq, np.float32))
    fq = np.ascontiguousarray(np.asarray(fq, np.float32))
    ps = np.ascontiguousarray(np.asarray(ps, np.float32))
    fs = np.ascontiguousarray(np.asarray(fs, np.float32))
    w = _prep_weights(params)

    if "nc" not in _CACHE:
        _CACHE["nc"] = _build_program([(k, v.shape) for k, v in w.items()])
    nc = _CACHE["nc"]

    in_maps = []
    for c in range(8):
        b, j = c // 4, c % 4
        m = dict(
            pq=pq[b], fq=fq[b], ps=ps[b], fs=fs[b],
            pq_sh=np.ascontiguousarray(pq[b][:, SHARD * j:SHARD * (j + 1)]),
            fq_sh=np.ascontiguousarray(fq[b][:, SHARD * j:SHARD * (j + 1)]),
        )
        m.update(w)
        in_maps.append(m)

    res = run_bass_kernel_spmd(nc, in_maps, core_ids=list(range(8)))
    out = np.zeros((B, C_IN, N), np.float32)
    for c in range(8):
        b, j = c // 4, c % 4
        out[b][:, SHARD * j:SHARD * (j + 1)] = res.results[c]["out"]
    return out


def timed_run(inputs_np):
    """Traced run returning HW exec time in ns (requires NTFF hook under axon)."""
    pq, fq, ps, fs = (inputs_np["pq"], inputs_np["fq"], inputs_np["ps"], inputs_np["fs"])
    w = _prep_weights(inputs_np["params"])
    nc = _CACHE["nc"]
    in_maps = []
    for c in range(8):
        b, j = c // 4, c % 4
        m = dict(pq=pq[b], fq=fq[b], ps=ps[b], fs=fs[b],
                 pq_sh=np.ascontiguousarray(pq[b][:, SHARD * j:SHARD * (j + 1)]),
                 fq_sh=np.ascontiguousarray(fq[b][:, SHARD * j:SHARD * (j + 1)]))
        m.update(w)
        in_maps.append(m)
    res = run_bass_kernel_spmd(nc, in_maps, core_ids=list(range(8)), trace=True)
    return res.exec_time_ns
